# revision 23
# baseline (speedup 1.0000x reference)
"""BilateralRotation Trainium2 kernel: out[b,c] = R1[c] @ wkv[b,c] @ R2[c],
R = Cayley(p) = (I - A)(I + A)^-1, A = 0.5(p - p^T).

Sharding: 8 NeuronCores, head-parallel — core k owns heads [4k, 4k+4) for all
512 batches (32 MB in / 32 MB out per core).

The tiny per-head rotations R1/R2 are computed on the HOST (fp64 numpy) and
shipped to each core pre-packed as the exact 128x128 block-diagonal stationary
matrices the device matmuls consume (bdl for MM1 parity-split, bdr for MM2).
This removes the on-device Newton-Schulz phase that previously serialized
~115us of pure compute in front of the DMA pipeline.

Device program per core (pure streaming):
  - input DMA in a folded layout (each partition holds two consecutive
    h-rows = 512B contiguous elements; 128 partitions span the core's 4
    heads with a single affine stride)
  - MM1 (Y = R1 X): two parity-split accumulating matmuls per head-pair,
    stationary = blockdiag of parity-sliced R1^T, moving = data, float32r
  - T1: PE 128x128 transposes (4 items each)
  - MM2 (Z^T = R2^T Y^T): stationary blockdiag(R2,R2), strided rhs
    gathering one head's columns, N=512, float32r
  - Z^T tiles for two consecutive batch-groups packed into one SBUF tile
    and dumped with a single DMA (16KB contiguous per partition); the host
    inverts the (fixed, known) index permutation while unsharding/
    concatenating the 8 shards.
"""

import sys
import types
from contextlib import ExitStack

import numpy as np

# ---------------------------------------------------------------------------
# TileContext patch: this walrus build accepts only ONE sync-wait per
# instruction; hoist extra waits onto nops inserted before the instruction.
# ---------------------------------------------------------------------------
import concourse.bass as bass
import concourse.tile as tile
from concourse.vector_clock import ScopedClock
from concourse import masks, mybir
from concourse.bass_utils import run_bass_kernel_spmd

WAIT_LIMIT = 1


def _hoist_extra_waits(nc, inst, hint):
    nops = []
    si = inst.sync_info
    if si is not None and len(si.on_wait) > WAIT_LIMIT:
        extras = si.on_wait[:-WAIT_LIMIT]
        del si.on_wait[:-WAIT_LIMIT]
        for w in extras:
            nop = nc.engines[inst.engine].nop(nofuse=True, hint=hint)
            nsi = nop.ins.sync_info
            if nsi is None:
                nop.ins.sync_info = mybir.SyncInfo(on_wait=[w], on_update=[])
            else:
                nsi.on_wait.append(w)
            nops.append(nop.ins)
    return nops


def _split_waits(nc):
    cur_list = nc.cur_bb.bb.instructions
    for f in nc.m.functions:
        for bb in f.blocks:
            orig = list(bb.instructions)
            if not any(i.sync_info and len(i.sync_info.on_wait) > WAIT_LIMIT
                       for i in orig):
                continue
            new_list = []
            for inst in orig:
                nops = _hoist_extra_waits(nc, inst, "split_wait")
                for nop in nops:
                    if cur_list and cur_list[-1] is nop:
                        cur_list.pop()
                    else:
                        cur_list.remove(nop)
                new_list.extend(nops)
                new_list.append(inst)
            bb.instructions[:] = new_list


def _drain_and_barrier(self, tick_clock, wait_clock):
    nc = self.nc
    _split_waits(nc)
    drain_inst = nc.sync.drain()
    wait_clock.add_sem_waits(drain_inst.ins,
                             ScopedClock({None: tick_clock.global_clock}))
    nops = _hoist_extra_waits(nc, drain_inst.ins, "drain_split_wait")
    if nops:
        insts = nc.cur_bb.bb.instructions
        di = insts.index(drain_inst.ins)
        insts.append(insts.pop(di))
    nc.all_engine_barrier()
    assert self.sems is not None
    popped = nc._tile_sem_poison_stack.pop()
    assert popped is self._sem_poison
    nc.clear_and_free_semaphores(list(self.sems.allocated().values()))
    nc.all_engine_barrier()


tile.TileContext._drain_and_barrier = _drain_and_barrier

# ---------------------------------------------------------------------------
# Program builder
# ---------------------------------------------------------------------------
dt = mybir.dt
F32 = dt.float32
F32R = dt.float32r
BF16 = dt.bfloat16

HPC = 4                     # heads per core
B = 512
H = W = 64
BSTRIDE = HPC * H * W
CSTRIDE = H * W
NG = 32                     # batch groups of 16
N_CORES = 8


def build(mm_f32r=True, in_bufs=8, out_bufs=2, mid_bufs=2):
    nc = bass.Bass("TRN2", target_bir_lowering=False, debug=False,
                   num_devices=N_CORES)
    mmdt = F32R if mm_f32r else F32
    wkv = nc.dram_tensor("wkv", [B, HPC, H, W], mmdt, kind="ExternalInput")
    bdl_d = nc.dram_tensor("bdl", [2, 2, 128, 128], mmdt,
                           kind="ExternalInput")
    bdr_d = nc.dram_tensor("bdr", [HPC, 128, 128], mmdt,
                           kind="ExternalInput")
    ident_d = nc.dram_tensor("ident", [128, 128], mmdt,
                             kind="ExternalInput")
    # Output batched two groups per DMA: 16KB contiguous per partition run
    # (vs 8KB) — half the packet/instruction count on the output stream.
    out = nc.dram_tensor("out_scr", [NG // 2, 128, 4096], F32,
                         kind="ExternalOutput")

    with tile.TileContext(nc) as tc, ExitStack() as ctx:
        const_pool = ctx.enter_context(tc.tile_pool(name="const", bufs=1))
        bd_pool = ctx.enter_context(tc.tile_pool(name="bd", bufs=1))
        io_pool = ctx.enter_context(tc.tile_pool(name="io", bufs=in_bufs))
        out_pool = ctx.enter_context(tc.tile_pool(name="outp", bufs=out_bufs))
        mid_pool = ctx.enter_context(tc.tile_pool(name="mid", bufs=mid_bufs))
        ps_pool = ctx.enter_context(
            tc.tile_pool(name="mainps", bufs=1, space="PSUM"))

        def load_xin(g):
            xin = io_pool.tile([128, 2048], mmdt, tag="xin")
            nc.sync.dma_start(
                xin[:], bass.AP(wkv, 16 * g * BSTRIDE,
                                [[128, 128], [BSTRIDE, 16], [1, 128]]))
            return xin

        # First input DMAs go to the queue head: the 32MB stream starts
        # immediately; the tiny stationary loads hide under it.
        PRE = 2
        pre_xin = [load_xin(g) for g in range(PRE)]

        # f32r identity shipped from host (gpsimd memset can't target f32r,
        # and a f32r identity makes T1 transposes 1.5 cyc/row vs 2.0 for f32).
        ident = const_pool.tile([128, 128], F32R, tag="ident")
        nc.sync.dma_start(ident[:], ident_d.ap())

        # Host-precomputed stationaries: MM1 parity blockdiags + MM2
        # head blockdiags. 8 x 64KB DMAs, negligible next to the 32MB stream.
        bdl = {}
        for P in range(2):
            for s in range(2):
                t = bd_pool.tile([128, 128], mmdt, tag=f"bdl{P}{s}")
                nc.sync.dma_start(t[:], bdl_d.ap()[P][s])
                bdl[(P, s)] = t
        bdr = []
        for c in range(HPC):
            t = bd_pool.tile([128, 128], mmdt, tag=f"bdr{c}")
            nc.sync.dma_start(t[:], bdr_d.ap()[c])
            bdr.append(t)

        # ---------------- main loop ----------------
        zsb = None
        for g in range(NG):
            xin = pre_xin[g] if g < PRE else load_xin(g)

            ysb = [mid_pool.tile([128, 1024], F32R, tag=f"ysb{P}",
                                 name=f"ysb{P}_{g}") for P in range(2)]
            for half in range(2):
                for P in range(2):
                    yps = ps_pool.tile([128, 512], F32, tag=f"mm1_{P}",
                                       bufs=2)
                    for s in range(2):
                        base = xin[64 * P:64 * P + 64,
                                   1024 * half + 64 * s:
                                   1024 * half + 64 * s + 64]
                        rhs = bass.AP(base.tensor, base.offset,
                                      [list(base.ap[0]), [128, 8], [1, 64]])
                        nc.tensor.matmul(
                            yps[:], bdl[(P, s)][64 * P:64 * P + 64, :], rhs,
                            start=(s == 0), stop=(s == 1),
                            tile_position=(64 * P, 0))
                    dstv = ysb[P][:, 512 * half:512 * half + 512]
                    if (half + P) % 2 == 0:
                        nc.vector.tensor_copy(dstv, yps[:])
                    else:
                        nc.scalar.copy(dstv, yps[:])

            ytsb = [mid_pool.tile([128, 1024], mmdt, tag=f"ytsb{P}",
                                  name=f"ytsb{P}_{g}") for P in range(2)]
            for P in range(2):
                for hp in range(2):
                    tps = ps_pool.tile([128, 512], F32R, tag="t1", bufs=2)
                    for q in range(4):
                        qq = 4 * hp + q
                        nc.tensor.transpose(
                            tps[:, 128 * q:128 * q + 128],
                            ysb[P][:, 128 * qq:128 * qq + 128], ident[:])
                    dstv = ytsb[P][:, 512 * hp:512 * hp + 512]
                    if (P + hp) % 2 == 0:
                        nc.vector.tensor_copy(dstv, tps[:])
                    else:
                        nc.scalar.copy(dstv, tps[:])

            if g % 2 == 0:
                zsb = out_pool.tile([128, 4096], F32, tag="zsb")
            zoff = 2048 * (g % 2)
            for c in range(HPC):
                P, hh = divmod(c, 2)
                zps = ps_pool.tile([128, 512], F32, tag="mm2", bufs=2)
                base = ytsb[P][:, 64 * hh:64 * hh + 64]
                rhs = bass.AP(base.tensor, base.offset,
                              [list(base.ap[0]), [128, 8], [1, 64]])
                nc.tensor.matmul(zps[:], bdr[c][:], rhs)
                dstv = zsb[:, zoff + 512 * c:zoff + 512 * c + 512]
                if c % 2 == 0:
                    nc.vector.tensor_copy(dstv, zps[:])
                else:
                    nc.scalar.copy(dstv, zps[:])

            # NOTE: output stays on the sync queue. Splitting it onto the
            # Activation HWDGE queue makes both DMA streams run concurrently,
            # which oversubscribes SBUF bandwidth and slows every compute
            # engine ~20% (measured 263us -> 310us).
            if g == NG - 2:
                nc.sync.dma_start(
                    bass.AP(out, (g // 2) * 128 * 4096,
                            [[4096, 128], [1, 2048]]),
                    zsb[:, 0:2048])
            elif g == NG - 1:
                nc.sync.dma_start(
                    bass.AP(out, (g // 2) * 128 * 4096 + 2048,
                            [[4096, 128], [1, 2048]]),
                    zsb[:, 2048:4096])
            elif g % 2 == 1:
                nc.sync.dma_start(
                    bass.AP(out, (g // 2) * 128 * 4096,
                            [[4096, 128], [1, 4096]]),
                    zsb[:])

    return nc


# ---------------------------------------------------------------------------
# Host-side rotation precompute
# ---------------------------------------------------------------------------
def _cayley_np(p):
    """R = (I - A)(I + A)^-1, A = 0.5(p - p^T); fp64 for exactness."""
    p = p.astype(np.float64)
    a = 0.5 * (p - np.swapaxes(p, -1, -2))
    eye = np.eye(p.shape[-1])
    inv = np.linalg.solve(eye[None] + a, np.broadcast_to(eye, a.shape))
    return (eye[None] - a) @ inv


def _pack_bd(r1, r2):
    """r1, r2: [HPC, 64, 64] fp32 -> (bdl [2,2,128,128], bdr [HPC,128,128]).

    bdl[P, s]: rows 64P+32hh..+32 x cols 64hh..+64 hold (R1[c][:, s::2])^T
    for c = 2P + hh — the parity-split MM1 stationary (stationary slice
    [64P:64P+64, :] maps moving partition (c, rowpair) -> output (hh, i)).
    bdr[c] = blockdiag(R2[c], R2[c]) for the N=512 MM2."""
    bdl = np.zeros((2, 2, 128, 128), dtype=np.float32)
    bdr = np.zeros((HPC, 128, 128), dtype=np.float32)
    for c in range(HPC):
        P, hh = divmod(c, 2)
        for s in range(2):
            blk = r1[c][:, s::2].T          # [32, 64] = [k2, i]
            r0 = 64 * P + 32 * hh
            bdl[P, s, r0:r0 + 32, 64 * hh:64 * hh + 64] = blk
        bdr[c, 0:64, 0:64] = r2[c]
        bdr[c, 64:128, 64:128] = r2[c]
    return bdl, bdr


def _unscramble(scr):
    """scr [NG//2, 128, 4096] -> [512, 4, 64, 64].
    Col-half u of a pair row is group 2*gp + u; within a group:
    scr[g, 64*bp + j, 512*h + 64*q + i] = Z[16g + 2q + bp, h][i, j]."""
    scr = scr.reshape(NG // 2, 128, 2, 2048).transpose(0, 2, 1, 3)
    a = scr.reshape(NG, 2, 64, HPC, 8, 64)      # g, bp, j, h, q, i
    a = a.transpose(0, 4, 1, 3, 5, 2)           # g, q, bp, h, i, j
    return np.ascontiguousarray(a.reshape(B, HPC, H, W))


_CACHED = {}


def _get_program():
    if "nc" not in _CACHED:
        _CACHED["nc"] = build()
    return _CACHED["nc"]


def kernel(wkv, p_left, p_right):
    wkv = np.ascontiguousarray(wkv, dtype=np.float32)
    p_left = np.ascontiguousarray(p_left, dtype=np.float32)
    p_right = np.ascontiguousarray(p_right, dtype=np.float32)
    assert wkv.shape == (B, 32, H, W), wkv.shape

    r1_all = _cayley_np(p_left).astype(np.float32)    # [32, 64, 64]
    r2_all = _cayley_np(p_right).astype(np.float32)   # [32, 64, 64]

    nc = _get_program()
    in_maps = []
    for k in range(N_CORES):
        sl = slice(HPC * k, HPC * k + HPC)
        bdl, bdr = _pack_bd(r1_all[sl], r2_all[sl])
        in_maps.append({
            "wkv": np.ascontiguousarray(wkv[:, sl]),
            "bdl": bdl,
            "bdr": bdr,
            "ident": np.eye(128, dtype=np.float32),
        })
    res = run_bass_kernel_spmd(nc, in_maps, list(range(N_CORES)))
    return np.concatenate(
        [_unscramble(np.asarray(res.results[k]["out_scr"]))
         for k in range(N_CORES)], axis=1)


# revision 24
# speedup vs baseline: 1.1241x; 1.1241x over previous
"""BilateralRotation Trainium2 kernel: out[b,c] = R1[c] @ wkv[b,c] @ R2[c],
R = Cayley(p) = (I - A)(I + A)^-1, A = 0.5(p - p^T).

Sharding: 8 NeuronCores, head-parallel — core k owns heads [4k, 4k+4) for all
512 batches (32 MB in / 32 MB out per core).

The tiny per-head rotations R1/R2 are computed on the HOST (fp64 numpy) and
shipped to each core pre-packed as the exact 128x128 block-diagonal stationary
matrices the device matmuls consume (bdl for MM1 parity-split, bdr for MM2).
This removes the on-device Newton-Schulz phase that previously serialized
~115us of pure compute in front of the DMA pipeline.

Device program per core (pure streaming):
  - input DMA in a folded layout (each partition holds two consecutive
    h-rows = 512B contiguous elements; 128 partitions span the core's 4
    heads with a single affine stride)
  - MM1 (Y = R1 X): two parity-split accumulating matmuls per head-pair,
    stationary = blockdiag of parity-sliced R1^T, moving = data, float32r
  - T1: PE 128x128 transposes (4 items each)
  - MM2 (Z^T = R2^T Y^T): stationary blockdiag(R2,R2), strided rhs
    gathering one head's columns, N=512, float32r
  - Z^T tiles for two consecutive batch-groups packed into one SBUF tile
    and dumped with a single DMA (16KB contiguous per partition); the host
    inverts the (fixed, known) index permutation while unsharding/
    concatenating the 8 shards.
"""

import sys
import types
from contextlib import ExitStack

import numpy as np

# ---------------------------------------------------------------------------
# TileContext patch: this walrus build accepts only ONE sync-wait per
# instruction; hoist extra waits onto nops inserted before the instruction.
# ---------------------------------------------------------------------------
import concourse.bass as bass
import concourse.tile as tile
from concourse.vector_clock import ScopedClock
from concourse import masks, mybir
from concourse.bass_utils import run_bass_kernel_spmd

WAIT_LIMIT = 1


def _hoist_extra_waits(nc, inst, hint):
    nops = []
    si = inst.sync_info
    if si is not None and len(si.on_wait) > WAIT_LIMIT:
        extras = si.on_wait[:-WAIT_LIMIT]
        del si.on_wait[:-WAIT_LIMIT]
        for w in extras:
            nop = nc.engines[inst.engine].nop(nofuse=True, hint=hint)
            nsi = nop.ins.sync_info
            if nsi is None:
                nop.ins.sync_info = mybir.SyncInfo(on_wait=[w], on_update=[])
            else:
                nsi.on_wait.append(w)
            nops.append(nop.ins)
    return nops


def _split_waits(nc):
    cur_list = nc.cur_bb.bb.instructions
    for f in nc.m.functions:
        for bb in f.blocks:
            orig = list(bb.instructions)
            if not any(i.sync_info and len(i.sync_info.on_wait) > WAIT_LIMIT
                       for i in orig):
                continue
            new_list = []
            for inst in orig:
                nops = _hoist_extra_waits(nc, inst, "split_wait")
                for nop in nops:
                    if cur_list and cur_list[-1] is nop:
                        cur_list.pop()
                    else:
                        cur_list.remove(nop)
                new_list.extend(nops)
                new_list.append(inst)
            bb.instructions[:] = new_list


def _drain_and_barrier(self, tick_clock, wait_clock):
    nc = self.nc
    _split_waits(nc)
    drain_inst = nc.sync.drain()
    wait_clock.add_sem_waits(drain_inst.ins,
                             ScopedClock({None: tick_clock.global_clock}))
    nops = _hoist_extra_waits(nc, drain_inst.ins, "drain_split_wait")
    if nops:
        insts = nc.cur_bb.bb.instructions
        di = insts.index(drain_inst.ins)
        insts.append(insts.pop(di))
    nc.all_engine_barrier()
    assert self.sems is not None
    popped = nc._tile_sem_poison_stack.pop()
    assert popped is self._sem_poison
    nc.clear_and_free_semaphores(list(self.sems.allocated().values()))
    nc.all_engine_barrier()


tile.TileContext._drain_and_barrier = _drain_and_barrier

# ---------------------------------------------------------------------------
# Program builder
# ---------------------------------------------------------------------------
dt = mybir.dt
F32 = dt.float32
F32R = dt.float32r
BF16 = dt.bfloat16

HPC = 4                     # heads per core
B = 512
H = W = 64
BSTRIDE = HPC * H * W
CSTRIDE = H * W
NG = 32                     # batch groups of 16
N_CORES = 8


def build(mm_f32r=True, in_bufs=8, out_bufs=2, mid_bufs=2):
    nc = bass.Bass("TRN2", target_bir_lowering=False, debug=False,
                   num_devices=N_CORES)
    mmdt = F32R if mm_f32r else F32
    wkv = nc.dram_tensor("wkv", [B, HPC, H, W], mmdt, kind="ExternalInput")
    bdl_d = nc.dram_tensor("bdl", [2, 2, 128, 128], mmdt,
                           kind="ExternalInput")
    bdr_d = nc.dram_tensor("bdr", [HPC, 128, 128], mmdt,
                           kind="ExternalInput")
    ident_d = nc.dram_tensor("ident", [128, 128], F32,
                             kind="ExternalInput")
    # Output batched two groups per DMA: 16KB contiguous per partition run
    # (vs 8KB) — half the packet/instruction count on the output stream.
    out = nc.dram_tensor("out_scr", [NG // 2, 128, 4096], F32,
                         kind="ExternalOutput")

    with tile.TileContext(nc) as tc, ExitStack() as ctx:
        const_pool = ctx.enter_context(tc.tile_pool(name="const", bufs=1))
        bd_pool = ctx.enter_context(tc.tile_pool(name="bd", bufs=1))
        io_pool = ctx.enter_context(tc.tile_pool(name="io", bufs=in_bufs))
        out_pool = ctx.enter_context(tc.tile_pool(name="outp", bufs=out_bufs))
        mid_pool = ctx.enter_context(tc.tile_pool(name="mid", bufs=mid_bufs))
        ps_pool = ctx.enter_context(
            tc.tile_pool(name="mainps", bufs=1, space="PSUM"))

        def load_xin(g):
            xin = io_pool.tile([128, 2048], mmdt, tag="xin")
            nc.sync.dma_start(
                xin[:], bass.AP(wkv, 16 * g * BSTRIDE,
                                [[128, 128], [BSTRIDE, 16], [1, 128]]))
            return xin

        # First input DMAs go to the queue head: the 32MB stream starts
        # immediately; the tiny stationary loads hide under it.
        PRE = 2
        pre_xin = [load_xin(g) for g in range(PRE)]

        # Identity shipped from host. NOTE: keep the whole T1 path in plain
        # f32 — both bf16 and f32r transposes measured SLOWER on this HW
        # (305us / 282us vs 246us) despite better cost-model rates.
        ident = const_pool.tile([128, 128], F32, tag="ident")
        nc.sync.dma_start(ident[:], ident_d.ap())

        # Host-precomputed stationaries: MM1 parity blockdiags + MM2
        # head blockdiags. 8 x 64KB DMAs, negligible next to the 32MB stream.
        bdl = {}
        for P in range(2):
            for s in range(2):
                t = bd_pool.tile([128, 128], mmdt, tag=f"bdl{P}{s}")
                nc.sync.dma_start(t[:], bdl_d.ap()[P][s])
                bdl[(P, s)] = t
        bdr = []
        for c in range(HPC):
            t = bd_pool.tile([128, 128], mmdt, tag=f"bdr{c}")
            nc.sync.dma_start(t[:], bdr_d.ap()[c])
            bdr.append(t)

        # ---------------- main loop ----------------
        zsb = None
        for g in range(NG):
            xin = pre_xin[g] if g < PRE else load_xin(g)

            ysb = [mid_pool.tile([128, 1024], F32, tag=f"ysb{P}",
                                 name=f"ysb{P}_{g}") for P in range(2)]
            for half in range(2):
                for P in range(2):
                    yps = ps_pool.tile([128, 512], F32, tag=f"mm1_{P}",
                                       bufs=2)
                    for s in range(2):
                        base = xin[64 * P:64 * P + 64,
                                   1024 * half + 64 * s:
                                   1024 * half + 64 * s + 64]
                        rhs = bass.AP(base.tensor, base.offset,
                                      [list(base.ap[0]), [128, 8], [1, 64]])
                        nc.tensor.matmul(
                            yps[:], bdl[(P, s)][64 * P:64 * P + 64, :], rhs,
                            start=(s == 0), stop=(s == 1),
                            tile_position=(64 * P, 0))
                    dstv = ysb[P][:, 512 * half:512 * half + 512]
                    if (half + P) % 2 == 0:
                        nc.vector.tensor_copy(dstv, yps[:])
                    else:
                        nc.scalar.copy(dstv, yps[:])

            ytsb = [mid_pool.tile([128, 1024], mmdt, tag=f"ytsb{P}",
                                  name=f"ytsb{P}_{g}") for P in range(2)]
            for P in range(2):
                for hp in range(2):
                    tps = ps_pool.tile([128, 512], F32, tag="t1", bufs=2)
                    for q in range(4):
                        qq = 4 * hp + q
                        nc.tensor.transpose(
                            tps[:, 128 * q:128 * q + 128],
                            ysb[P][:, 128 * qq:128 * qq + 128], ident[:])
                    dstv = ytsb[P][:, 512 * hp:512 * hp + 512]
                    if (P + hp) % 2 == 0:
                        nc.vector.tensor_copy(dstv, tps[:])
                    else:
                        nc.scalar.copy(dstv, tps[:])

            if g % 2 == 0:
                zsb = out_pool.tile([128, 4096], F32, tag="zsb")
            zoff = 2048 * (g % 2)
            for c in range(HPC):
                P, hh = divmod(c, 2)
                zps = ps_pool.tile([128, 512], F32, tag="mm2", bufs=2)
                base = ytsb[P][:, 64 * hh:64 * hh + 64]
                rhs = bass.AP(base.tensor, base.offset,
                              [list(base.ap[0]), [128, 8], [1, 64]])
                nc.tensor.matmul(zps[:], bdr[c][:], rhs)
                dstv = zsb[:, zoff + 512 * c:zoff + 512 * c + 512]
                if c % 2 == 0:
                    nc.vector.tensor_copy(dstv, zps[:])
                else:
                    nc.scalar.copy(dstv, zps[:])

            # NOTE: output stays on the sync queue. Splitting it onto the
            # Activation HWDGE queue makes both DMA streams run concurrently,
            # which oversubscribes SBUF bandwidth and slows every compute
            # engine ~20% (measured 263us -> 310us).
            if g == NG - 2:
                nc.sync.dma_start(
                    bass.AP(out, (g // 2) * 128 * 4096,
                            [[4096, 128], [1, 2048]]),
                    zsb[:, 0:2048])
            elif g == NG - 1:
                nc.sync.dma_start(
                    bass.AP(out, (g // 2) * 128 * 4096 + 2048,
                            [[4096, 128], [1, 2048]]),
                    zsb[:, 2048:4096])
            elif g % 2 == 1:
                nc.sync.dma_start(
                    bass.AP(out, (g // 2) * 128 * 4096,
                            [[4096, 128], [1, 4096]]),
                    zsb[:])

    return nc


# ---------------------------------------------------------------------------
# Host-side rotation precompute
# ---------------------------------------------------------------------------
def _cayley_np(p):
    """R = (I - A)(I + A)^-1, A = 0.5(p - p^T); fp64 for exactness."""
    p = p.astype(np.float64)
    a = 0.5 * (p - np.swapaxes(p, -1, -2))
    eye = np.eye(p.shape[-1])
    inv = np.linalg.solve(eye[None] + a, np.broadcast_to(eye, a.shape))
    return (eye[None] - a) @ inv


def _pack_bd(r1, r2):
    """r1, r2: [HPC, 64, 64] fp32 -> (bdl [2,2,128,128], bdr [HPC,128,128]).

    bdl[P, s]: rows 64P+32hh..+32 x cols 64hh..+64 hold (R1[c][:, s::2])^T
    for c = 2P + hh — the parity-split MM1 stationary (stationary slice
    [64P:64P+64, :] maps moving partition (c, rowpair) -> output (hh, i)).
    bdr[c] = blockdiag(R2[c], R2[c]) for the N=512 MM2."""
    bdl = np.zeros((2, 2, 128, 128), dtype=np.float32)
    bdr = np.zeros((HPC, 128, 128), dtype=np.float32)
    for c in range(HPC):
        P, hh = divmod(c, 2)
        for s in range(2):
            blk = r1[c][:, s::2].T          # [32, 64] = [k2, i]
            r0 = 64 * P + 32 * hh
            bdl[P, s, r0:r0 + 32, 64 * hh:64 * hh + 64] = blk
        bdr[c, 0:64, 0:64] = r2[c]
        bdr[c, 64:128, 64:128] = r2[c]
    return bdl, bdr


def _unscramble(scr):
    """scr [NG//2, 128, 4096] -> [512, 4, 64, 64].
    Col-half u of a pair row is group 2*gp + u; within a group:
    scr[g, 64*bp + j, 512*h + 64*q + i] = Z[16g + 2q + bp, h][i, j]."""
    scr = scr.reshape(NG // 2, 128, 2, 2048).transpose(0, 2, 1, 3)
    a = scr.reshape(NG, 2, 64, HPC, 8, 64)      # g, bp, j, h, q, i
    a = a.transpose(0, 4, 1, 3, 5, 2)           # g, q, bp, h, i, j
    return np.ascontiguousarray(a.reshape(B, HPC, H, W))


_CACHED = {}


def _get_program():
    if "nc" not in _CACHED:
        _CACHED["nc"] = build()
    return _CACHED["nc"]


def kernel(wkv, p_left, p_right):
    wkv = np.ascontiguousarray(wkv, dtype=np.float32)
    p_left = np.ascontiguousarray(p_left, dtype=np.float32)
    p_right = np.ascontiguousarray(p_right, dtype=np.float32)
    assert wkv.shape == (B, 32, H, W), wkv.shape

    r1_all = _cayley_np(p_left).astype(np.float32)    # [32, 64, 64]
    r2_all = _cayley_np(p_right).astype(np.float32)   # [32, 64, 64]

    nc = _get_program()
    in_maps = []
    for k in range(N_CORES):
        sl = slice(HPC * k, HPC * k + HPC)
        bdl, bdr = _pack_bd(r1_all[sl], r2_all[sl])
        in_maps.append({
            "wkv": np.ascontiguousarray(wkv[:, sl]),
            "bdl": bdl,
            "bdr": bdr,
            "ident": np.eye(128, dtype=np.float32),
        })
    res = run_bass_kernel_spmd(nc, in_maps, list(range(N_CORES)))
    return np.concatenate(
        [_unscramble(np.asarray(res.results[k]["out_scr"]))
         for k in range(N_CORES)], axis=1)


# revision 25
# speedup vs baseline: 1.2170x; 1.0826x over previous
"""BilateralRotation Trainium2 kernel: out[b,c] = R1[c] @ wkv[b,c] @ R2[c],
R = Cayley(p) = (I - A)(I + A)^-1, A = 0.5(p - p^T).

Sharding: 8 NeuronCores, head-parallel — core k owns heads [4k, 4k+4) for all
512 batches (32 MB in / 32 MB out per core).

The tiny per-head rotations R1/R2 are computed on the HOST (fp64 numpy) and
shipped to each core pre-packed as the exact 128x128 block-diagonal stationary
matrices the device matmuls consume (bdl for MM1 parity-split, bdr for MM2).
This removes the on-device Newton-Schulz phase that previously serialized
~115us of pure compute in front of the DMA pipeline.

Device program per core (pure streaming):
  - input DMA in a folded layout (each partition holds two consecutive
    h-rows = 512B contiguous elements; 128 partitions span the core's 4
    heads with a single affine stride)
  - MM1 (Y = R1 X): two parity-split accumulating matmuls per head-pair,
    stationary = blockdiag of parity-sliced R1^T, moving = data, float32r
  - T1: PE 128x128 transposes (4 items each)
  - MM2 (Z^T = R2^T Y^T): stationary blockdiag(R2,R2), strided rhs
    gathering one head's columns, N=512, float32r
  - Z^T tiles for two consecutive batch-groups packed into one SBUF tile
    and dumped with a single DMA (16KB contiguous per partition); the host
    inverts the (fixed, known) index permutation while unsharding/
    concatenating the 8 shards.
"""

import sys
import types
from contextlib import ExitStack

import numpy as np

# ---------------------------------------------------------------------------
# TileContext patch: this walrus build accepts only ONE sync-wait per
# instruction; hoist extra waits onto nops inserted before the instruction.
# ---------------------------------------------------------------------------
import concourse.bass as bass
import concourse.tile as tile
from concourse.vector_clock import ScopedClock
from concourse import masks, mybir
from concourse.bass_utils import run_bass_kernel_spmd

WAIT_LIMIT = 1


def _hoist_extra_waits(nc, inst, hint):
    nops = []
    si = inst.sync_info
    if si is not None and len(si.on_wait) > WAIT_LIMIT:
        extras = si.on_wait[:-WAIT_LIMIT]
        del si.on_wait[:-WAIT_LIMIT]
        for w in extras:
            nop = nc.engines[inst.engine].nop(nofuse=True, hint=hint)
            nsi = nop.ins.sync_info
            if nsi is None:
                nop.ins.sync_info = mybir.SyncInfo(on_wait=[w], on_update=[])
            else:
                nsi.on_wait.append(w)
            nops.append(nop.ins)
    return nops


def _split_waits(nc):
    cur_list = nc.cur_bb.bb.instructions
    for f in nc.m.functions:
        for bb in f.blocks:
            orig = list(bb.instructions)
            if not any(i.sync_info and len(i.sync_info.on_wait) > WAIT_LIMIT
                       for i in orig):
                continue
            new_list = []
            for inst in orig:
                nops = _hoist_extra_waits(nc, inst, "split_wait")
                for nop in nops:
                    if cur_list and cur_list[-1] is nop:
                        cur_list.pop()
                    else:
                        cur_list.remove(nop)
                new_list.extend(nops)
                new_list.append(inst)
            bb.instructions[:] = new_list


def _drain_and_barrier(self, tick_clock, wait_clock):
    nc = self.nc
    _split_waits(nc)
    drain_inst = nc.sync.drain()
    wait_clock.add_sem_waits(drain_inst.ins,
                             ScopedClock({None: tick_clock.global_clock}))
    nops = _hoist_extra_waits(nc, drain_inst.ins, "drain_split_wait")
    if nops:
        insts = nc.cur_bb.bb.instructions
        di = insts.index(drain_inst.ins)
        insts.append(insts.pop(di))
    nc.all_engine_barrier()
    assert self.sems is not None
    popped = nc._tile_sem_poison_stack.pop()
    assert popped is self._sem_poison
    nc.clear_and_free_semaphores(list(self.sems.allocated().values()))
    nc.all_engine_barrier()


tile.TileContext._drain_and_barrier = _drain_and_barrier

# ---------------------------------------------------------------------------
# Program builder
# ---------------------------------------------------------------------------
dt = mybir.dt
F32 = dt.float32
F32R = dt.float32r
BF16 = dt.bfloat16

HPC = 4                     # heads per core
B = 512
H = W = 64
BSTRIDE = HPC * H * W
CSTRIDE = H * W
NG = 32                     # batch groups of 16
N_CORES = 8


def build(mm_f32r=True, in_bufs=4, out_bufs=2, mid_bufs=2):
    nc = bass.Bass("TRN2", target_bir_lowering=False, debug=False,
                   num_devices=N_CORES)
    mmdt = F32R if mm_f32r else F32
    wkv = nc.dram_tensor("wkv", [B, HPC, H, W], mmdt, kind="ExternalInput")
    bdl_d = nc.dram_tensor("bdl", [2, 2, 128, 128], mmdt,
                           kind="ExternalInput")
    bdr_d = nc.dram_tensor("bdr", [HPC, 128, 128], mmdt,
                           kind="ExternalInput")
    ident_d = nc.dram_tensor("ident", [128, 128], F32,
                             kind="ExternalInput")
    # Output batched two groups per DMA: 16KB contiguous per partition run
    # (vs 8KB) — half the packet/instruction count on the output stream.
    out = nc.dram_tensor("out_scr", [NG // 2, 128, 4096], F32,
                         kind="ExternalOutput")

    with tile.TileContext(nc) as tc, ExitStack() as ctx:
        const_pool = ctx.enter_context(tc.tile_pool(name="const", bufs=1))
        bd_pool = ctx.enter_context(tc.tile_pool(name="bd", bufs=1))
        io_pool = ctx.enter_context(tc.tile_pool(name="io", bufs=in_bufs))
        out_pool = ctx.enter_context(tc.tile_pool(name="outp", bufs=out_bufs))
        mid_pool = ctx.enter_context(tc.tile_pool(name="mid", bufs=mid_bufs))
        ps_pool = ctx.enter_context(
            tc.tile_pool(name="mainps", bufs=1, space="PSUM"))

        def load_xin(gp):
            # One DMA per PAIR of batch-groups (4096 descriptors): fewer
            # DGE restarts and sync-engine dispatches on the input stream.
            xin = io_pool.tile([128, 4096], mmdt, tag="xin")
            nc.sync.dma_start(
                xin[:], bass.AP(wkv, 32 * gp * BSTRIDE,
                                [[128, 128], [BSTRIDE, 32], [1, 128]]))
            return xin

        # First input DMA goes to the queue head: the 32MB stream starts
        # immediately; the tiny stationary loads hide under it.
        pre_xin = load_xin(0)

        # Identity shipped from host. NOTE: keep the whole T1 path in plain
        # f32 — both bf16 and f32r transposes measured SLOWER on this HW
        # (305us / 282us vs 246us) despite better cost-model rates.
        ident = const_pool.tile([128, 128], F32, tag="ident")
        nc.sync.dma_start(ident[:], ident_d.ap())

        # Host-precomputed stationaries: MM1 parity blockdiags + MM2
        # head blockdiags. 8 x 64KB DMAs, negligible next to the 32MB stream.
        bdl = {}
        for P in range(2):
            for s in range(2):
                t = bd_pool.tile([128, 128], mmdt, tag=f"bdl{P}{s}")
                nc.sync.dma_start(t[:], bdl_d.ap()[P][s])
                bdl[(P, s)] = t
        bdr = []
        for c in range(HPC):
            t = bd_pool.tile([128, 128], mmdt, tag=f"bdr{c}")
            nc.sync.dma_start(t[:], bdr_d.ap()[c])
            bdr.append(t)

        # ---------------- main loop ----------------
        zsb = None
        xin2 = None
        for g in range(NG):
            if g % 2 == 0:
                xin2 = pre_xin if g == 0 else load_xin(g // 2)
            xoff = 2048 * (g % 2)

            ysb = [mid_pool.tile([128, 1024], F32, tag=f"ysb{P}",
                                 name=f"ysb{P}_{g}") for P in range(2)]
            for half in range(2):
                for P in range(2):
                    yps = ps_pool.tile([128, 512], F32, tag=f"mm1_{P}",
                                       bufs=2)
                    for s in range(2):
                        base = xin2[64 * P:64 * P + 64,
                                    xoff + 1024 * half + 64 * s:
                                    xoff + 1024 * half + 64 * s + 64]
                        rhs = bass.AP(base.tensor, base.offset,
                                      [list(base.ap[0]), [128, 8], [1, 64]])
                        nc.tensor.matmul(
                            yps[:], bdl[(P, s)][64 * P:64 * P + 64, :], rhs,
                            start=(s == 0), stop=(s == 1),
                            tile_position=(64 * P, 0))
                    dstv = ysb[P][:, 512 * half:512 * half + 512]
                    if (half + P) % 2 == 0:
                        nc.vector.tensor_copy(dstv, yps[:])
                    else:
                        nc.scalar.copy(dstv, yps[:])

            ytsb = [mid_pool.tile([128, 1024], mmdt, tag=f"ytsb{P}",
                                  name=f"ytsb{P}_{g}") for P in range(2)]
            for P in range(2):
                for hp in range(2):
                    tps = ps_pool.tile([128, 512], F32, tag="t1", bufs=2)
                    for q in range(4):
                        qq = 4 * hp + q
                        nc.tensor.transpose(
                            tps[:, 128 * q:128 * q + 128],
                            ysb[P][:, 128 * qq:128 * qq + 128], ident[:])
                    dstv = ytsb[P][:, 512 * hp:512 * hp + 512]
                    if (P + hp) % 2 == 0:
                        nc.vector.tensor_copy(dstv, tps[:])
                    else:
                        nc.scalar.copy(dstv, tps[:])

            if g % 2 == 0:
                zsb = out_pool.tile([128, 4096], F32, tag="zsb")
            zoff = 2048 * (g % 2)
            for c in range(HPC):
                P, hh = divmod(c, 2)
                zps = ps_pool.tile([128, 512], F32, tag="mm2", bufs=2)
                base = ytsb[P][:, 64 * hh:64 * hh + 64]
                rhs = bass.AP(base.tensor, base.offset,
                              [list(base.ap[0]), [128, 8], [1, 64]])
                nc.tensor.matmul(zps[:], bdr[c][:], rhs)
                dstv = zsb[:, zoff + 512 * c:zoff + 512 * c + 512]
                if c % 2 == 0:
                    nc.vector.tensor_copy(dstv, zps[:])
                else:
                    nc.scalar.copy(dstv, zps[:])

            # NOTE: output stays on the sync queue. Splitting it onto the
            # Activation HWDGE queue makes both DMA streams run concurrently,
            # which oversubscribes SBUF bandwidth and slows every compute
            # engine ~20% (measured 263us -> 310us).
            if g == NG - 2:
                nc.sync.dma_start(
                    bass.AP(out, (g // 2) * 128 * 4096,
                            [[4096, 128], [1, 2048]]),
                    zsb[:, 0:2048])
            elif g == NG - 1:
                nc.sync.dma_start(
                    bass.AP(out, (g // 2) * 128 * 4096 + 2048,
                            [[4096, 128], [1, 2048]]),
                    zsb[:, 2048:4096])
            elif g % 2 == 1:
                nc.sync.dma_start(
                    bass.AP(out, (g // 2) * 128 * 4096,
                            [[4096, 128], [1, 4096]]),
                    zsb[:])

    return nc


# ---------------------------------------------------------------------------
# Host-side rotation precompute
# ---------------------------------------------------------------------------
def _cayley_np(p):
    """R = (I - A)(I + A)^-1, A = 0.5(p - p^T); fp64 for exactness."""
    p = p.astype(np.float64)
    a = 0.5 * (p - np.swapaxes(p, -1, -2))
    eye = np.eye(p.shape[-1])
    inv = np.linalg.solve(eye[None] + a, np.broadcast_to(eye, a.shape))
    return (eye[None] - a) @ inv


def _pack_bd(r1, r2):
    """r1, r2: [HPC, 64, 64] fp32 -> (bdl [2,2,128,128], bdr [HPC,128,128]).

    bdl[P, s]: rows 64P+32hh..+32 x cols 64hh..+64 hold (R1[c][:, s::2])^T
    for c = 2P + hh — the parity-split MM1 stationary (stationary slice
    [64P:64P+64, :] maps moving partition (c, rowpair) -> output (hh, i)).
    bdr[c] = blockdiag(R2[c], R2[c]) for the N=512 MM2."""
    bdl = np.zeros((2, 2, 128, 128), dtype=np.float32)
    bdr = np.zeros((HPC, 128, 128), dtype=np.float32)
    for c in range(HPC):
        P, hh = divmod(c, 2)
        for s in range(2):
            blk = r1[c][:, s::2].T          # [32, 64] = [k2, i]
            r0 = 64 * P + 32 * hh
            bdl[P, s, r0:r0 + 32, 64 * hh:64 * hh + 64] = blk
        bdr[c, 0:64, 0:64] = r2[c]
        bdr[c, 64:128, 64:128] = r2[c]
    return bdl, bdr


def _unscramble(scr):
    """scr [NG//2, 128, 4096] -> [512, 4, 64, 64].
    Col-half u of a pair row is group 2*gp + u; within a group:
    scr[g, 64*bp + j, 512*h + 64*q + i] = Z[16g + 2q + bp, h][i, j]."""
    scr = scr.reshape(NG // 2, 128, 2, 2048).transpose(0, 2, 1, 3)
    a = scr.reshape(NG, 2, 64, HPC, 8, 64)      # g, bp, j, h, q, i
    a = a.transpose(0, 4, 1, 3, 5, 2)           # g, q, bp, h, i, j
    return np.ascontiguousarray(a.reshape(B, HPC, H, W))


_CACHED = {}


def _get_program():
    if "nc" not in _CACHED:
        _CACHED["nc"] = build()
    return _CACHED["nc"]


def kernel(wkv, p_left, p_right):
    wkv = np.ascontiguousarray(wkv, dtype=np.float32)
    p_left = np.ascontiguousarray(p_left, dtype=np.float32)
    p_right = np.ascontiguousarray(p_right, dtype=np.float32)
    assert wkv.shape == (B, 32, H, W), wkv.shape

    r1_all = _cayley_np(p_left).astype(np.float32)    # [32, 64, 64]
    r2_all = _cayley_np(p_right).astype(np.float32)   # [32, 64, 64]

    nc = _get_program()
    in_maps = []
    for k in range(N_CORES):
        sl = slice(HPC * k, HPC * k + HPC)
        bdl, bdr = _pack_bd(r1_all[sl], r2_all[sl])
        in_maps.append({
            "wkv": np.ascontiguousarray(wkv[:, sl]),
            "bdl": bdl,
            "bdr": bdr,
            "ident": np.eye(128, dtype=np.float32),
        })
    res = run_bass_kernel_spmd(nc, in_maps, list(range(N_CORES)))
    return np.concatenate(
        [_unscramble(np.asarray(res.results[k]["out_scr"]))
         for k in range(N_CORES)], axis=1)
